# revision 2
# baseline (speedup 1.0000x reference)
"""Trainium2 Bass kernel for nn_AsymmetricProjectedLinear.

Reference computes out = x @ W_large^T with
    W_large = (A_out @ B_out) @ W_small @ (A_in @ B_in)^T    [4096, 4096]

We never materialize W_large. Factored:
    M   = B_in @ W_small^T @ B_out^T            [64, 64]   (tiny)
    out = ((x @ A_in) @ M) @ A_out^T            [4096t, 4096]

Sharding: tokens (B*S = 4096) split 512/core across 8 cores. W_small is
sharded by columns (128/core) for the M prework; the per-core partial M
[64, 64] is summed with a tiny AllReduce that hides under the x stream.
Host work is layout-only (transpose/pack/slice); all FLOPs run on the
NeuronCores.

Per-core device pipeline (T = 512 local tokens, i_c = local 128-col W slice):
  prework:  G_c = B_out @ W_small[:, i_c]       [64, 128]   (PE, K=j)
            G_c^T via PE transpose               [128, 64]
            M_c = B_in[:, i_c] @ G_c^T           [64, 64]
            M   = AllReduce_add(M_c)                        (hidden)
  stage 1:  u1T = A_in^T @ xT                    [64, T]     (PE, K=d 4096)
  stage 2:  t2T = lhsT=M, rhs=u1T                [64, T]
  stage 5:  out[t, o] : lhsT=t2T slice, rhs=A_out^T, row-packed 2x

All matmul operands are fed in natural [K-on-partitions] layout via
host-side packing, so no on-chip transposes of x are needed.
"""

import os

import numpy as np

import concourse.bass as bass
import concourse.mybir as mybir
import concourse.tile as tile
from concourse import bacc
from concourse.bass_utils import run_bass_kernel_spmd

N_CORES = 8
Bsz, S, D = 2, 2048, 4096
TOK = Bsz * S          # 4096 tokens
T = TOK // N_CORES     # 512 tokens per core
RANK = 64
DS = 1024              # d_small

F32 = mybir.dt.float32
# float32r streams the PE at full rate (1 cyc/row for N>=256) vs fp32's 4
# cyc/row, with slightly reduced internal precision. Toggle for A/B.
USE_F32R = os.environ.get("KERNEL_F32R", "1") == "1"
MM_DT = mybir.dt.float32r if USE_F32R else mybir.dt.float32

_nc_cache = {}


def build():
    key = (MM_DT,)
    if key in _nc_cache:
        return _nc_cache[key]
    nc = bacc.Bacc("TRN2", target_bir_lowering=False, debug=False,
                   num_devices=N_CORES)

    # Inputs, all pre-packed on host into direct SBUF layouts.
    x_p = nc.dram_tensor("x_p", [128, 32 * T], MM_DT, kind="ExternalInput")
    a_in_p = nc.dram_tensor("a_in_p", [128, 32 * RANK], MM_DT, kind="ExternalInput")
    a_outT = nc.dram_tensor("a_outT", [RANK, D], MM_DT, kind="ExternalInput")
    b_inT_s128 = nc.dram_tensor("b_inT_s128", [128, RANK], MM_DT, kind="ExternalInput")
    b_outT_p = nc.dram_tensor("b_outT_p", [128, 8 * RANK], MM_DT, kind="ExternalInput")
    w_slice = nc.dram_tensor("w_slice", [128, 8 * 128], MM_DT, kind="ExternalInput")
    ident = nc.dram_tensor("ident", [RANK, RANK], MM_DT, kind="ExternalInput")
    out = nc.dram_tensor("out", [T, D], F32, kind="ExternalOutput")

    with tile.TileContext(nc) as tc:
        with (
            tc.tile_pool(name="const", bufs=1) as cpool,
            tc.tile_pool(name="xin", bufs=4) as xpool,
            tc.tile_pool(name="outp", bufs=2) as opool,
            tc.tile_pool(name="dram", bufs=1, space="DRAM") as dpool,
            tc.tile_pool(name="ps_a", bufs=2, space="PSUM") as ps_a,
            tc.tile_pool(name="ps_o", bufs=6, space="PSUM") as ps_o,
        ):
            # ---- x stream first on the Sync HWDGE ring ---------------
            x_tiles = []
            for c in range(8):
                xt = xpool.tile([128, 4 * T], MM_DT)
                nc.sync.dma_start(
                    out=xt[:, :], in_=x_p.ap()[:, c * 4 * T:(c + 1) * 4 * T]
                )
                x_tiles.append(xt)

            # ---- weights on the Scalar HWDGE ring --------------------
            a_in_s = cpool.tile([128, 32 * RANK], MM_DT)
            nc.scalar.dma_start(out=a_in_s[:, :], in_=a_in_p.ap())
            w_s = cpool.tile([128, 8 * 128], MM_DT)
            nc.scalar.dma_start(out=w_s[:, :], in_=w_slice.ap())
            b_outT_s = cpool.tile([128, 8 * RANK], MM_DT)
            nc.scalar.dma_start(out=b_outT_s[:, :], in_=b_outT_p.ap())
            b_inT_s = cpool.tile([128, RANK], MM_DT)
            nc.scalar.dma_start(out=b_inT_s[:, :], in_=b_inT_s128.ap())
            ident_s = cpool.tile([RANK, RANK], MM_DT)
            nc.scalar.dma_start(out=ident_s[:, :], in_=ident.ap())
            # A_out^T duplicated into both partition halves for row-packed
            # stage-5 matmuls.
            a_outT_s = cpool.tile([128, D], MM_DT)
            nc.scalar.dma_start(out=a_outT_s[:RANK, :], in_=a_outT.ap())
            nc.scalar.dma_start(out=a_outT_s[RANK:, :], in_=a_outT.ap())

            # ---- prework: G_c = B_out @ W_small[:, i_c]  [64, 128] ---
            g_ps = ps_a.tile([RANK, 128], F32, tag="ps_small")
            for j in range(8):
                nc.tensor.matmul(
                    g_ps[:, :],
                    b_outT_s[:, j * RANK:(j + 1) * RANK],
                    w_s[:, j * 128:(j + 1) * 128],
                    start=(j == 0), stop=(j == 7),
                )
            g_s = cpool.tile([RANK, 128], MM_DT)
            nc.vector.tensor_copy(g_s[:, :], g_ps[:, :])

            # ---- G_c^T via PE transpose  [128, 64] -------------------
            gt_ps = ps_a.tile([128, RANK], MM_DT, tag="ps_small")
            nc.tensor.transpose(gt_ps[:, :], g_s[:, :], ident_s[:, :])
            gT_s = cpool.tile([128, RANK], MM_DT)
            nc.vector.tensor_copy(gT_s[:, :], gt_ps[:, :])

            # ---- M_c = B_in[:, i_c] @ G_c^T  [64, 64] ----------------
            m_ps = ps_a.tile([RANK, RANK], F32, tag="ps_small")
            nc.tensor.matmul(
                m_ps[:, :], b_inT_s[:, :], gT_s[:, :], start=True, stop=True,
            )
            m_part = cpool.tile([RANK, RANK], F32)
            nc.vector.tensor_copy(m_part[:, :], m_ps[:, :])

            # ---- AllReduce M across the 8 cores (16KB; hides under x)
            cc_in = dpool.tile([RANK, RANK], F32)
            cc_out = dpool.tile([RANK, RANK], F32)
            nc.gpsimd.dma_start(out=cc_in[:, :], in_=m_part[:, :])
            nc.gpsimd.collective_compute(
                "AllReduce",
                mybir.AluOpType.add,
                replica_groups=[list(range(N_CORES))],
                ins=[cc_in.opt()],
                outs=[cc_out.opt()],
            )
            m_f32 = cpool.tile([RANK, RANK], F32)
            nc.gpsimd.dma_start(out=m_f32[:, :], in_=cc_out[:, :])
            m_s = cpool.tile([RANK, RANK], MM_DT)
            nc.vector.tensor_copy(m_s[:, :], m_f32[:, :])

            # ---- stage 1: u1T = A_in^T @ xT  [64, T] ----------------
            u1_ps = ps_a.tile([RANK, T], F32, tag="ps_small")
            for m in range(32):
                xt = x_tiles[m // 4]
                sl = m % 4
                nc.tensor.matmul(
                    u1_ps[:, :],
                    a_in_s[:, m * RANK:(m + 1) * RANK],
                    xt[:, sl * T:(sl + 1) * T],
                    start=(m == 0), stop=(m == 31),
                )
            u1_s = cpool.tile([RANK, T], MM_DT)
            nc.vector.tensor_copy(u1_s[:, :], u1_ps[:, :])

            # ---- stage 2: t2T = (u1 M)^T = lhsT=M, rhs=u1T ----------
            t2_ps = ps_a.tile([RANK, T], F32, tag="ps_small")
            nc.tensor.matmul(
                t2_ps[:, :], m_s[:, :], u1_s[:, :], start=True, stop=True,
            )
            # duplicate t2T into both partition halves for row packing
            t2_s = cpool.tile([128, T], MM_DT)
            nc.vector.tensor_copy(t2_s[:RANK, :], t2_ps[:, :])
            nc.scalar.copy(t2_s[RANK:, :], t2_ps[:, :])

            # ---- stage 5: out[t, o] = t2 @ A_out^T, row-packed 2x ---
            # t-slices ts and ts+2 run concurrently in PE row halves.
            for half in range(2):          # (ts0, ts1) = (half, half+2)
                ts0, ts1 = half, half + 2
                o_t0 = opool.tile([128, D], F32, tag="o_lo")
                o_t1 = opool.tile([128, D], F32, tag="o_hi")
                for o in range(8):
                    po0 = ps_o.tile([128, 512], F32, tag="ps_out")
                    po1 = ps_o.tile([128, 512], F32, tag="ps_out")
                    nc.tensor.matmul(
                        po0[:, :],
                        t2_s[:RANK, ts0 * 128:(ts0 + 1) * 128],
                        a_outT_s[:RANK, o * 512:(o + 1) * 512],
                        start=True, stop=True,
                    )
                    nc.tensor.matmul(
                        po1[:, :],
                        t2_s[RANK:, ts1 * 128:(ts1 + 1) * 128],
                        a_outT_s[RANK:, o * 512:(o + 1) * 512],
                        start=True, stop=True,
                        tile_position=(64, 0),
                    )
                    nc.vector.tensor_copy(o_t0[:, o * 512:(o + 1) * 512], po0[:, :])
                    nc.scalar.copy(o_t1[:, o * 512:(o + 1) * 512], po1[:, :])
                    if o % 2 == 1:
                        for t_idx, o_t in ((ts0, o_t0), (ts1, o_t1)):
                            nc.sync.dma_start(
                                out=out.ap()[t_idx * 128:(t_idx + 1) * 128,
                                             (o - 1) * 512:(o + 1) * 512],
                                in_=o_t[:, (o - 1) * 512:(o + 1) * 512],
                            )

    nc.compile()
    _nc_cache[key] = nc
    return nc


def _prep_in_maps(x, W_small, A_out, B_out, A_in, B_in):
    f = np.float32
    x2 = np.asarray(x, dtype=f).reshape(TOK, D)
    a_in_p = np.ascontiguousarray(
        np.asarray(A_in, f).reshape(32, 128, RANK).transpose(1, 0, 2)
    ).reshape(128, 32 * RANK)
    a_outT = np.ascontiguousarray(np.asarray(A_out, f).T)
    b_inT = np.asarray(B_in, f).T                      # [1024, 64]
    b_outT_p = np.ascontiguousarray(
        np.asarray(B_out, f).T.reshape(8, 128, RANK).transpose(1, 0, 2)
    ).reshape(128, 8 * RANK)
    w = np.asarray(W_small, f)                         # [1024, 1024]
    ident = np.eye(RANK, dtype=f)

    shared = {
        "a_in_p": a_in_p, "a_outT": a_outT, "b_outT_p": b_outT_p,
        "ident": ident,
    }
    in_maps = []
    for c in range(N_CORES):
        xs = x2[c * T:(c + 1) * T, :]            # [T, 4096]
        # -> [4096, T] -> packed 32 d-tiles [128, T] along free dim
        xp = np.ascontiguousarray(
            xs.T.reshape(32, 128, T).transpose(1, 0, 2)
        ).reshape(128, 32 * T)
        isl = slice(c * 128, (c + 1) * 128)
        # W_small[:, i_c] packed as 8 j-tiles [128, 128] along free dim
        w_sl = np.ascontiguousarray(
            w[:, isl].reshape(8, 128, 128).transpose(1, 0, 2)
        ).reshape(128, 8 * 128)
        in_maps.append({
            "x_p": xp,
            "w_slice": w_sl,
            "b_inT_s128": np.ascontiguousarray(b_inT[isl, :]),
            **shared,
        })
    return in_maps


def _run(inputs, trace=False):
    nc = build()
    in_maps = _prep_in_maps(**inputs)
    res = run_bass_kernel_spmd(
        nc, in_maps, core_ids=list(range(N_CORES)), trace=trace
    )
    out = np.concatenate(
        [res.results[c]["out"] for c in range(N_CORES)], axis=0
    ).reshape(Bsz, S, D)
    return out, res


def kernel(**inputs) -> np.ndarray:
    out, _ = _run(inputs, trace=False)
    return out


# revision 4
# speedup vs baseline: 1.4019x; 1.4019x over previous
"""Trainium2 Bass kernel for nn_AsymmetricProjectedLinear.

Reference computes out = x @ W_large^T with
    W_large = (A_out @ B_out) @ W_small @ (A_in @ B_in)^T    [4096, 4096]

We never materialize W_large. Factored:
    M   = B_in @ W_small^T @ B_out^T            [64, 64]   (tiny)
    out = ((x @ A_in) @ M) @ A_out^T            [4096t, 4096]

Sharding: tokens (B*S = 4096) split 512/core across 8 cores; weights
replicated (a 16KB AllReduce for M was tried and costs ~50us wall on this
runtime -- far above its nominal floor -- so every core redundantly
computes M from the full W_small instead). Host work is layout-only
(transpose/pack/slice); all FLOPs run on the NeuronCores.

Per-core device pipeline (T = 512 local tokens):
  prework:  G = B_out @ W_small                 [64, 1024]  (PE, K=j)
            G^T via PE transpose                 [1024, 64]
            M = B_in @ G^T                       [64, 64]
  stage 1:  u1T = A_in^T @ xT                    [64, T]     (PE, K=d 4096)
  stage 2:  t2T = lhsT=M, rhs=u1T, col-packed 2x -> both partition halves
  stage 5:  out[t, o] : lhsT=t2T slice, rhs=A_out^T, row-packed 2x

All matmul operands are fed in natural [K-on-partitions] layout via
host-side packing, so no on-chip transposes of x are needed.
"""

import os

import numpy as np

import concourse.bass as bass
import concourse.mybir as mybir
import concourse.tile as tile
from concourse import bacc
from concourse.bass_utils import run_bass_kernel_spmd

N_CORES = 8
Bsz, S, D = 2, 2048, 4096
TOK = Bsz * S          # 4096 tokens
T = TOK // N_CORES     # 512 tokens per core
RANK = 64
DS = 1024              # d_small

F32 = mybir.dt.float32
# float32r streams the PE at full rate (1 cyc/row for N>=256) vs fp32's 4
# cyc/row, with slightly reduced internal precision. Toggle for A/B.
USE_F32R = os.environ.get("KERNEL_F32R", "1") == "1"
MM_DT = mybir.dt.float32r if USE_F32R else mybir.dt.float32

_nc_cache = {}


def build():
    key = (MM_DT,)
    if key in _nc_cache:
        return _nc_cache[key]
    nc = bacc.Bacc("TRN2", target_bir_lowering=False, debug=False,
                   num_devices=N_CORES)

    # Inputs, all pre-packed on host into direct SBUF layouts.
    x_p = nc.dram_tensor("x_p", [128, 32 * T], MM_DT, kind="ExternalInput")
    a_in_p = nc.dram_tensor("a_in_p", [128, 32 * RANK], MM_DT, kind="ExternalInput")
    a_outT = nc.dram_tensor("a_outT", [RANK, D], MM_DT, kind="ExternalInput")
    b_inT_p = nc.dram_tensor("b_inT_p", [128, 8 * RANK], MM_DT, kind="ExternalInput")
    b_outT_p = nc.dram_tensor("b_outT_p", [128, 8 * RANK], MM_DT, kind="ExternalInput")
    w_p = nc.dram_tensor("w_p", [128, 8 * DS], MM_DT, kind="ExternalInput")
    ident = nc.dram_tensor("ident", [RANK, RANK], MM_DT, kind="ExternalInput")
    out = nc.dram_tensor("out", [T, D], F32, kind="ExternalOutput")

    with tile.TileContext(nc) as tc:
        with (
            tc.tile_pool(name="const", bufs=1) as cpool,
            tc.tile_pool(name="xin", bufs=4) as xpool,
            tc.tile_pool(name="outp", bufs=3) as opool,
            tc.tile_pool(name="ps_a", bufs=2, space="PSUM") as ps_a,
            tc.tile_pool(name="ps_o", bufs=6, space="PSUM") as ps_o,
        ):
            # ---- x stream first on the Sync HWDGE ring ---------------
            x_tiles = []
            for c in range(8):
                xt = xpool.tile([128, 4 * T], MM_DT)
                nc.sync.dma_start(
                    out=xt[:, :], in_=x_p.ap()[:, c * 4 * T:(c + 1) * 4 * T]
                )
                x_tiles.append(xt)

            # ---- weights on the Scalar HWDGE ring --------------------
            a_in_s = cpool.tile([128, 32 * RANK], MM_DT)
            nc.scalar.dma_start(out=a_in_s[:, :], in_=a_in_p.ap())
            w_s = cpool.tile([128, 8 * DS], MM_DT)
            nc.scalar.dma_start(out=w_s[:, :], in_=w_p.ap())
            b_outT_s = cpool.tile([128, 8 * RANK], MM_DT)
            nc.scalar.dma_start(out=b_outT_s[:, :], in_=b_outT_p.ap())
            b_inT_s = cpool.tile([128, 8 * RANK], MM_DT)
            nc.scalar.dma_start(out=b_inT_s[:, :], in_=b_inT_p.ap())
            ident_s = cpool.tile([RANK, RANK], MM_DT)
            nc.scalar.dma_start(out=ident_s[:, :], in_=ident.ap())
            # A_out^T duplicated into both partition halves for row-packed
            # stage-5 matmuls.
            a_outT_s = cpool.tile([128, D], MM_DT)
            nc.scalar.dma_start(out=a_outT_s[:RANK, :], in_=a_outT.ap())
            nc.gpsimd.dma_start(out=a_outT_s[RANK:, :], in_=a_outT_s[:RANK, :])

            # ---- prework: G = B_out @ W_small  [64, 1024] ------------
            g_s = cpool.tile([RANK, DS], MM_DT)
            for h in range(2):
                g_ps = ps_a.tile([RANK, 512], F32, tag="ps_small")
                for j in range(8):
                    nc.tensor.matmul(
                        g_ps[:, :],
                        b_outT_s[:, j * RANK:(j + 1) * RANK],
                        w_s[:, j * DS + h * 512: j * DS + (h + 1) * 512],
                        start=(j == 0), stop=(j == 7),
                    )
                nc.vector.tensor_copy(g_s[:, h * 512:(h + 1) * 512], g_ps[:, :])

            # ---- G^T via PE transpose  [1024, 64] --------------------
            gT_s = cpool.tile([128, 8 * RANK], MM_DT)
            for it in range(8):
                gt_ps = ps_a.tile([128, RANK], MM_DT, tag="ps_small")
                nc.tensor.transpose(
                    gt_ps[:, :], g_s[:, it * 128:(it + 1) * 128], ident_s[:, :]
                )
                nc.vector.tensor_copy(
                    gT_s[:, it * RANK:(it + 1) * RANK], gt_ps[:, :]
                )

            # ---- M = B_in @ G^T  [64, 64] ----------------------------
            m_ps = ps_a.tile([RANK, RANK], F32, tag="ps_small")
            for it in range(8):
                nc.tensor.matmul(
                    m_ps[:, :],
                    b_inT_s[:, it * RANK:(it + 1) * RANK],
                    gT_s[:, it * RANK:(it + 1) * RANK],
                    start=(it == 0), stop=(it == 7),
                )
            m_s = cpool.tile([RANK, RANK], MM_DT)
            nc.vector.tensor_copy(m_s[:, :], m_ps[:, :])

            # ---- stage 1: u1T = A_in^T @ xT  [64, T] ----------------
            u1_ps = ps_a.tile([RANK, T], F32, tag="ps_small")
            for m in range(32):
                xt = x_tiles[m // 4]
                sl = m % 4
                nc.tensor.matmul(
                    u1_ps[:, :],
                    a_in_s[:, m * RANK:(m + 1) * RANK],
                    xt[:, sl * T:(sl + 1) * T],
                    start=(m == 0), stop=(m == 31),
                )
            u1_s = cpool.tile([RANK, T], MM_DT)
            nc.vector.tensor_copy(u1_s[:, :], u1_ps[:, :])

            # ---- stage 2: t2T = (u1 M)^T; duplicate into both halves
            # (row-packed stage 5 needs t2T at partitions 0-63 and 64-127;
            # cross-partition moves need a DMA, DVE lanes are fixed)
            t2_ps = ps_a.tile([RANK, T], F32, tag="ps_small")
            nc.tensor.matmul(
                t2_ps[:, :], m_s[:, :], u1_s[:, :], start=True, stop=True,
            )
            t2_s = cpool.tile([128, T], MM_DT)
            nc.vector.tensor_copy(t2_s[:RANK, :], t2_ps[:, :])
            nc.gpsimd.dma_start(out=t2_s[RANK:, :], in_=t2_s[:RANK, :])

            # ---- stage 5: out[t, o] = t2 @ A_out^T, row-packed 2x ---
            # t-slices ts and ts+2 run concurrently in PE row halves.
            for half in range(2):          # (ts0, ts1) = (half, half+2)
                ts0, ts1 = half, half + 2
                for op in range(4):        # o-pairs
                    o_t0 = opool.tile([128, 1024], F32, tag="o_lo")
                    o_t1 = opool.tile([128, 1024], F32, tag="o_hi")
                    for oi in range(2):
                        o = op * 2 + oi
                        po0 = ps_o.tile([128, 512], F32, tag="ps_out")
                        po1 = ps_o.tile([128, 512], F32, tag="ps_out")
                        nc.tensor.matmul(
                            po0[:, :],
                            t2_s[:RANK, ts0 * 128:(ts0 + 1) * 128],
                            a_outT_s[:RANK, o * 512:(o + 1) * 512],
                            start=True, stop=True,
                        )
                        nc.tensor.matmul(
                            po1[:, :],
                            t2_s[RANK:, ts1 * 128:(ts1 + 1) * 128],
                            a_outT_s[RANK:, o * 512:(o + 1) * 512],
                            start=True, stop=True,
                        )
                        nc.vector.tensor_copy(
                            o_t0[:, oi * 512:(oi + 1) * 512], po0[:, :])
                        nc.scalar.copy(
                            o_t1[:, oi * 512:(oi + 1) * 512], po1[:, :])
                    for t_idx, o_t in ((ts0, o_t0), (ts1, o_t1)):
                        nc.sync.dma_start(
                            out=out.ap()[t_idx * 128:(t_idx + 1) * 128,
                                         op * 1024:(op + 1) * 1024],
                            in_=o_t[:, :],
                        )

    nc.compile()
    _nc_cache[key] = nc
    return nc


def _prep_in_maps(x, W_small, A_out, B_out, A_in, B_in):
    f = np.float32
    x2 = np.asarray(x, dtype=f).reshape(TOK, D)
    a_in_p = np.ascontiguousarray(
        np.asarray(A_in, f).reshape(32, 128, RANK).transpose(1, 0, 2)
    ).reshape(128, 32 * RANK)
    a_outT = np.ascontiguousarray(np.asarray(A_out, f).T)
    b_inT_p = np.ascontiguousarray(
        np.asarray(B_in, f).T.reshape(8, 128, RANK).transpose(1, 0, 2)
    ).reshape(128, 8 * RANK)
    b_outT_p = np.ascontiguousarray(
        np.asarray(B_out, f).T.reshape(8, 128, RANK).transpose(1, 0, 2)
    ).reshape(128, 8 * RANK)
    w_p = np.ascontiguousarray(
        np.asarray(W_small, f).reshape(8, 128, DS).transpose(1, 0, 2)
    ).reshape(128, 8 * DS)
    ident = np.eye(RANK, dtype=f)

    shared = {
        "a_in_p": a_in_p, "a_outT": a_outT, "b_inT_p": b_inT_p,
        "b_outT_p": b_outT_p, "w_p": w_p, "ident": ident,
    }
    in_maps = []
    for c in range(N_CORES):
        xs = x2[c * T:(c + 1) * T, :]            # [T, 4096]
        # -> [4096, T] -> packed 32 d-tiles [128, T] along free dim
        xp = np.ascontiguousarray(
            xs.T.reshape(32, 128, T).transpose(1, 0, 2)
        ).reshape(128, 32 * T)
        in_maps.append({"x_p": xp, **shared})
    return in_maps


def _run(inputs, trace=False):
    nc = build()
    in_maps = _prep_in_maps(**inputs)
    res = run_bass_kernel_spmd(
        nc, in_maps, core_ids=list(range(N_CORES)), trace=trace
    )
    out = np.concatenate(
        [res.results[c]["out"] for c in range(N_CORES)], axis=0
    ).reshape(Bsz, S, D)
    return out, res


def kernel(**inputs) -> np.ndarray:
    out, _ = _run(inputs, trace=False)
    return out
